# revision 1
# baseline (speedup 1.0000x reference)
"""Longformer-style sliding-chunk self-attention for Trainium2 (Bass/Tile).

Problem: B=2, T=4096, E=768, H=12 heads (head dim 64), window chunk W=256.
  q = (x @ wq.T)/8, k = x @ wk.T, v = x @ wv.T  (per head)
  scores: each chunk of 256 queries attends to [prev, cur, next] chunks
  (3*256 = 768 keys, zero-padded at sequence ends, with triangular masks on
  the pad blocks), softmax over the 768 window, then probs @ V.

Sharding: 8 cores = 2 batches x 4 head-groups of 3 heads. Each core gets
x[b].T (pre-transposed on host), per-head weight slices (transposed on
host, with the 1/8 query scale folded into wq), and produces
out[b, :, g*192:(g+1)*192].

Per-core kernel (all layouts chosen so no on-chip transposes are needed):
  - Q.T, K.T computed in [head_dim, T] layout (PSUM out of matmuls with
    weight slices as the stationary operand, x.T streaming).
  - V computed in natural [T, head_dim] layout (x.T tiles stationary,
    wv.T streaming), stored with a ones-column appended (V_aug) so the
    P@V matmul also produces the softmax denominator for free.
  - scores are computed TRANSPOSED: S.T[key, q] = K.T_tile.T @ Q.T_chunk,
    softmax uses exp WITHOUT max subtraction (scores ~ N(0,1), max < ~7,
    exp is safe in fp32) so no partition-dim reduction is ever needed.
  - P@V: out[q, s] = expS.T_tile.T @ V_aug accumulated over the 6 key
    tiles of the window; column 64 is the denominator; divide via
    reciprocal + tensor_scalar multiply.
  - boundary chunks: the zero-padded prev/next blocks have score 0, so
    exp(0)*mask = mask; the 0/1 mask tiles (precomputed on host) are used
    directly as the "expS" stationary operand with a zeros+ones V_aug pad
    tile, which also fixes the denominator. No masking work in the kernel.
"""

import math

import numpy as np

B, T, E, H, WIN = 2, 4096, 768, 12, 256
S = 64            # head dim
NH = 3            # heads per core
ET = E // 128     # 6 e-tiles
TT = T // 128     # 32 t-tiles
C = T // WIN      # 16 chunks
NCORES = 8
NCH = 8           # 512-wide column chunks for the projections
PROJN = T // NCH  # 512


def _build_module(loop_n=None):
    """Build + compile the per-core Bass module. Same program on all cores."""
    from contextlib import ExitStack

    import concourse.mybir as mybir
    from concourse import bacc
    from concourse.tile import TileContext

    fp32 = mybir.dt.float32
    Exp = mybir.ActivationFunctionType.Exp

    nc = bacc.Bacc("TRN2", target_bir_lowering=False, debug=False,
                   num_devices=NCORES)
    xT = nc.dram_tensor("xT", [E, T], fp32, kind="ExternalInput")
    wqk = nc.dram_tensor("wqk", [E, NH, 128], fp32, kind="ExternalInput")
    wv = nc.dram_tensor("wv", [E, NH, S], fp32, kind="ExternalInput")
    masks = nc.dram_tensor("masks", [128, 4, WIN], fp32, kind="ExternalInput")
    out = nc.dram_tensor("out", [T, NH * S], fp32, kind="ExternalOutput")

    def emit(tc, ctx):
        singles = ctx.enter_context(tc.tile_pool(name="singles", bufs=1))
        qk_pool = ctx.enter_context(tc.tile_pool(name="qk", bufs=1))
        st_pool = ctx.enter_context(tc.tile_pool(name="st", bufs=2, space="PSUM"))
        pv_pool = ctx.enter_context(tc.tile_pool(name="pv", bufs=2, space="PSUM"))
        ex_pool = ctx.enter_context(tc.tile_pool(name="ex", bufs=2))
        o_pool = ctx.enter_context(tc.tile_pool(name="o", bufs=4))
        sm_pool = ctx.enter_context(tc.tile_pool(name="sm", bufs=4))

        # ---- persistent SBUF tensors ----
        xt = singles.tile([128, ET, T], fp32)            # x[b].T   96KB/part
        wqk_sb = singles.tile([128, ET, NH, 128], fp32)  # 9KB/part
        wv_sb = singles.tile([128, ET, NH, S], fp32)     # 4.5KB/part
        mask_sb = singles.tile([128, 4, WIN], fp32)      # 4KB/part
        v3 = singles.tile([128, TT, NH, S + 1], fp32)    # V_aug  24.4KB/part
        vpad = singles.tile([128, S + 1], fp32)

        # ---- input loads ----
        xT_r = xT.ap().rearrange("(a p) t -> a p t", p=128)
        for et in range(ET):
            for tq in range(4):
                nc.sync.dma_start(out=xt[:, et, tq * 1024:(tq + 1) * 1024],
                                  in_=xT_r[et][:, tq * 1024:(tq + 1) * 1024])
        nc.sync.dma_start(out=wqk_sb[:],
                          in_=wqk.ap().rearrange("(a p) g m -> p a g m", p=128))
        nc.sync.dma_start(out=wv_sb[:],
                          in_=wv.ap().rearrange("(a p) g m -> p a g m", p=128))
        nc.sync.dma_start(out=mask_sb[:], in_=masks.ap())
        nc.vector.memset(vpad[:], 0.0)
        nc.vector.memset(vpad[:, S:S + 1], 1.0)
        nc.vector.memset(v3[:, :, :, S:S + 1], 1.0)

        # ---- V projection, all heads: V[t, s] (+ ones col) ----
        for tt in range(TT):
            pvv = pv_pool.tile([128, NH * S], fp32, tag="pv")
            for et in range(ET):
                nc.tensor.matmul(pvv[:],
                                 xt[:, et, tt * 128:(tt + 1) * 128],
                                 wv_sb[:, et, :, :],
                                 start=(et == 0), stop=(et == ET - 1))
            nc.vector.tensor_copy(
                out=v3[:, tt, :, 0:S],
                in_=pvv.rearrange("p (g s) -> p g s", g=NH))

        # ---- per-head: Q.T/K.T projection, then attention ----
        for g in range(NH):
            qt = qk_pool.tile([64, T], fp32, tag="qt")
            kt = qk_pool.tile([64, T], fp32, tag="kt")
            for nch in range(NCH):
                sl = slice(nch * PROJN, (nch + 1) * PROJN)
                psq = st_pool.tile([64, PROJN], fp32, tag="st")
                for et in range(ET):
                    nc.tensor.matmul(psq[:], wqk_sb[:, et, g, 0:64],
                                     xt[:, et, sl],
                                     start=(et == 0), stop=(et == ET - 1))
                nc.scalar.copy(out=qt[:, sl], in_=psq[:])
                psk = st_pool.tile([64, PROJN], fp32, tag="st")
                for et in range(ET):
                    nc.tensor.matmul(psk[:], wqk_sb[:, et, g, 64:128],
                                     xt[:, et, sl],
                                     start=(et == 0), stop=(et == ET - 1))
                nc.vector.tensor_copy(out=kt[:, sl], in_=psk[:])

            for c in range(C):
                lo = 2 if c == 0 else 0        # first valid window key-tile
                hi = 4 if c == C - 1 else 6    # one past last valid
                stp = st_pool.tile([128, 6, WIN], fp32, tag="st")
                for w_i in range(lo, hi):
                    gk = (c - 1) * 2 + w_i
                    nc.tensor.matmul(stp[:, w_i, :],
                                     kt[:, gk * 128:(gk + 1) * 128],
                                     qt[:, c * WIN:(c + 1) * WIN],
                                     start=True, stop=True)
                ex = ex_pool.tile([128, 6, WIN], fp32)
                nc.scalar.activation(out=ex[:, lo:hi, :], in_=stp[:, lo:hi, :],
                                     func=Exp)
                pv = pv_pool.tile([128, 2, S + 1], fp32, tag="pv")
                for qh in range(2):
                    qsl = slice(qh * 128, (qh + 1) * 128)
                    for w_i in range(6):
                        if w_i < lo:
                            lhs = mask_sb[:, w_i, qsl]
                            rhs = vpad[:]
                        elif w_i >= hi:
                            lhs = mask_sb[:, 2 + (w_i - 4), qsl]
                            rhs = vpad[:]
                        else:
                            gk = (c - 1) * 2 + w_i
                            lhs = ex[:, w_i, qsl]
                            rhs = v3[:, gk, g, :]
                        nc.tensor.matmul(pv[:, qh, :], lhs, rhs,
                                         start=(w_i == 0), stop=(w_i == 5))
                rc = sm_pool.tile([128, 2, 1], fp32)
                nc.vector.reciprocal(rc[:], pv[:, :, S:S + 1])
                ob = o_pool.tile([128, 2, S], fp32)
                for qh in range(2):
                    nc.vector.tensor_scalar_mul(ob[:, qh, :], pv[:, qh, 0:S],
                                                rc[:, qh, 0:1])
                    nc.sync.dma_start(
                        out=out.ap()[c * WIN + qh * 128:c * WIN + (qh + 1) * 128,
                                     g * S:(g + 1) * S],
                        in_=ob[:, qh, :])

    with TileContext(nc) as tc:
        with ExitStack() as ctx:
            if loop_n is None:
                emit(tc, ctx)
            else:
                with tc.For_i(0, loop_n, 1):
                    emit(tc, ctx)
    nc.compile()
    return nc


def _make_masks():
    """0/1 multiplicative masks for the zero-padded prev/next blocks, in
    expS.T layout [key_within_tile, q]. Slots 0,1: chunk-0 prev tiles;
    slots 2,3: chunk-15 next tiles."""
    m = np.ones((128, 4, WIN), dtype=np.float32)
    p = np.arange(128)[:, None]
    q = np.arange(WIN)[None, :]
    for kt in range(2):
        k = kt * 128 + p
        m[:, kt, :] = np.where(q < WIN - k, 0.0, 1.0)
    for et in range(2):
        kn = et * 128 + p
        m[:, 2 + et, :] = np.where(q >= (WIN - 1) - kn, 0.0, 1.0)
    return m


def _prep_inputs(x, wq, wk, wv):
    """Host-side shard prep: per-core input dicts."""
    masks = _make_masks()
    xTb = [np.ascontiguousarray(x[b].T) for b in range(B)]
    wqs = wq.astype(np.float32) * np.float32(1.0 / math.sqrt(S))
    in_maps = []
    for core in range(NCORES):
        b, grp = divmod(core, 4)
        h0 = grp * NH
        wqk_np = np.empty((E, NH, 128), dtype=np.float32)
        wv_np = np.empty((E, NH, S), dtype=np.float32)
        for g in range(NH):
            h = h0 + g
            wqk_np[:, g, 0:64] = wqs[h * S:(h + 1) * S, :].T
            wqk_np[:, g, 64:128] = wk[h * S:(h + 1) * S, :].T
            wv_np[:, g, :] = wv[h * S:(h + 1) * S, :].T
        in_maps.append({"xT": xTb[b], "wqk": wqk_np, "wv": wv_np,
                        "masks": masks})
    return in_maps


class _Runner:
    """Compile once; execute many times via PJRT across the 8 cores."""

    def __init__(self, loop_n=None):
        import jax
        import concourse.mybir as mybir
        from concourse import bass2jax
        from jax.sharding import Mesh, PartitionSpec
        from jax.experimental.shard_map import shard_map

        self.jax = jax
        nc = _build_module(loop_n=loop_n)
        self.nc = nc
        bass2jax.install_neuronx_cc_hook()

        partition_name = (nc.partition_id_tensor.name
                          if nc.partition_id_tensor else None)
        in_names, out_names, out_avals = [], [], []
        for alloc in nc.m.functions[0].allocations:
            if not isinstance(alloc, mybir.MemoryLocationSet):
                continue
            name = alloc.memorylocations[0].name
            if alloc.kind == "ExternalInput":
                if name != partition_name:
                    in_names.append(name)
            elif alloc.kind == "ExternalOutput":
                out_names.append(name)
                out_avals.append(jax.core.ShapedArray(
                    tuple(alloc.tensor_shape), mybir.dt.np(alloc.dtype)))
        self.in_names = in_names
        self.out_names = out_names
        n_params = len(in_names)
        n_outs = len(out_names)
        self.out_avals = out_avals
        in_names_all = list(in_names) + list(out_names)
        if partition_name:
            in_names_all.append(partition_name)

        def _body(*args):
            operands = list(args)
            if partition_name is not None:
                operands.append(bass2jax.partition_id_tensor())
            outs = bass2jax._bass_exec_p.bind(
                *operands, out_avals=tuple(out_avals),
                in_names=tuple(in_names_all), out_names=tuple(out_names),
                lowering_input_output_aliases=(),
                sim_require_finite=True, sim_require_nnan=True, nc=nc)
            return tuple(outs)

        devices = jax.devices()[:NCORES]
        mesh = Mesh(np.asarray(devices), ("core",))
        self._fn = jax.jit(
            shard_map(_body, mesh=mesh,
                      in_specs=(PartitionSpec("core"),) * (n_params + n_outs),
                      out_specs=(PartitionSpec("core"),) * n_outs,
                      check_rep=False),
            keep_unused=True)

    def put_args(self, in_maps):
        concat_in = [np.concatenate([m[nm] for m in in_maps], axis=0)
                     for nm in self.in_names]
        concat_zero = [np.zeros((NCORES * a.shape[0], *a.shape[1:]), a.dtype)
                       for a in self.out_avals]
        return [self.jax.device_put(a) for a in concat_in + concat_zero]

    def run(self, args):
        res = self.jax.block_until_ready(self._fn(*args))
        return [np.asarray(r) for r in res]


_RUNNER = None


def kernel(x, wq, wk, wv):
    global _RUNNER
    x = np.asarray(x, dtype=np.float32)
    wq = np.asarray(wq, dtype=np.float32)
    wk = np.asarray(wk, dtype=np.float32)
    wv = np.asarray(wv, dtype=np.float32)
    if _RUNNER is None:
        _RUNNER = _Runner()
    in_maps = _prep_inputs(x, wq, wk, wv)
    args = _RUNNER.put_args(in_maps)
    outs = _RUNNER.run(args)
    o = outs[0].reshape(NCORES, T, NH * S)
    full = np.empty((B, T, E), dtype=np.float32)
    for core in range(NCORES):
        b, grp = divmod(core, 4)
        full[b, :, grp * NH * S:(grp + 1) * NH * S] = o[core]
    return full


# revision 11
# speedup vs baseline: 3.7956x; 3.7956x over previous
"""Longformer-style sliding-chunk self-attention for Trainium2 (Bass/Tile).

Problem: B=2, T=4096, E=768, H=12 heads (head dim 64), window chunk W=256.
  q = (x @ wq.T)/8, k = x @ wk.T, v = x @ wv.T  (per head)
  scores: each chunk of 256 queries attends to [prev, cur, next] chunks
  (3*256 = 768 keys, zero-padded at sequence ends, with triangular masks on
  the pad blocks), softmax over the 768 window, then probs @ V.

Sharding: 8 cores = 2 batches x 4 head-groups of 3 heads. Each core gets
x[b].T (pre-transposed on host), per-head weight slices (transposed on
host, with the 1/8 query scale folded into wq), and produces
out[b, :, g*192:(g+1)*192].

Per-core kernel (all layouts chosen so no on-chip transposes are needed):
  - Q.T, K.T computed in [head_dim, T] layout (PSUM out of matmuls with
    weight slices as the stationary operand, x.T streaming).
  - V computed in natural [T, head_dim] layout (x.T tiles stationary,
    wv.T streaming), stored with a ones-column appended (V_aug) so the
    P@V matmul also produces the softmax denominator for free.
  - scores are computed TRANSPOSED: S.T[key, q] = K.T_tile.T @ Q.T_chunk,
    softmax uses exp WITHOUT max subtraction (scores ~ N(0,1), max < ~7,
    exp is safe in fp32) so no partition-dim reduction is ever needed.
  - P@V: out[q, s] = expS.T_tile.T @ V_aug accumulated over the 6 key
    tiles of the window; column 64 is the denominator; divide via
    reciprocal + tensor_scalar multiply.
  - boundary chunks: the zero-padded prev/next blocks have score 0, so
    exp(0)*mask = mask; the 0/1 mask tiles (precomputed on host) are used
    directly as the "expS" stationary operand with a zeros+ones V_aug pad
    tile, which also fixes the denominator. No masking work in the kernel.
"""

import math

import numpy as np

B, T, E, H, WIN = 2, 4096, 768, 12, 256
S = 64            # head dim
NH = 3            # heads per core
ET = E // 128     # 6 e-tiles
TT = T // 128     # 32 t-tiles
C = T // WIN      # 16 chunks
NCORES = 8
NCH = 8           # 512-wide column chunks for the projections
PROJN = T // NCH  # 512


def _build_module(loop_n=None):
    """Build + compile the per-core Bass module. Same program on all cores."""
    from contextlib import ExitStack

    import concourse.mybir as mybir
    from concourse import bacc
    from concourse.tile import TileContext

    fp32 = mybir.dt.float32
    fp32r = mybir.dt.float32r
    Exp = mybir.ActivationFunctionType.Exp

    nc = bacc.Bacc("TRN2", target_bir_lowering=False, debug=False,
                   num_devices=NCORES)
    xT = nc.dram_tensor("xT", [E, T], fp32, kind="ExternalInput")
    wqk = nc.dram_tensor("wqk", [E, NH, 128], fp32, kind="ExternalInput")
    # wv is padded to 4*S=256 columns (last 64 zero) so the V projection's
    # moving dim is 256, which lets float32r run at 1 cycle/row.
    wv = nc.dram_tensor("wv", [E, 4 * S], fp32, kind="ExternalInput")
    masks = nc.dram_tensor("masks", [128, 4, WIN], fp32, kind="ExternalInput")
    out = nc.dram_tensor("out", [T, NH * S], fp32, kind="ExternalOutput")

    def emit(tc, ctx):
        singles = ctx.enter_context(tc.tile_pool(name="singles", bufs=1))
        qk_pool = ctx.enter_context(tc.tile_pool(name="qk", bufs=1))
        st_pool = ctx.enter_context(tc.tile_pool(name="st", bufs=2, space="PSUM"))
        pv_pool = ctx.enter_context(tc.tile_pool(name="pv", bufs=2, space="PSUM"))
        ex_pool = ctx.enter_context(tc.tile_pool(name="ex", bufs=2))
        o_pool = ctx.enter_context(tc.tile_pool(name="o", bufs=4))
        sm_pool = ctx.enter_context(tc.tile_pool(name="sm", bufs=4))

        # ---- persistent SBUF tensors ----
        xt = singles.tile([128, ET, T], fp32)            # x[b].T   96KB/part
        wqk_sb = singles.tile([128, ET, NH, 128], fp32)  # 9KB/part
        wv_sb = singles.tile([128, ET, 4 * S], fp32)     # 6KB/part
        mask_sb = singles.tile([128, 4, WIN], fp32)      # 4KB/part
        v3 = singles.tile([128, TT, NH, S + 1], fp32)    # V_aug  24.4KB/part
        vpad = singles.tile([128, S + 1], fp32)

        # ---- input loads ----
        # xt/wqk/wv feed float32r matmuls: the BIR verifier requires their
        # producers to emit float32r, so the loads are bitcast on both sides
        # (same 4-byte values; the PE does the hi/lo bf16 split at load).
        xT_r = xT.ap().bitcast(fp32r).rearrange("(a p) t -> a p t", p=128)
        for et in range(ET):
            for tq in range(4):
                nc.sync.dma_start(out=xt[:, et, tq * 1024:(tq + 1) * 1024].bitcast(fp32r),
                                  in_=xT_r[et][:, tq * 1024:(tq + 1) * 1024])
        nc.sync.dma_start(out=wqk_sb[:].bitcast(fp32r),
                          in_=wqk.ap().bitcast(fp32r).rearrange("(a p) g m -> p a g m", p=128))
        nc.sync.dma_start(out=wv_sb[:].bitcast(fp32r),
                          in_=wv.ap().bitcast(fp32r).rearrange("(a p) m -> p a m", p=128))
        nc.sync.dma_start(out=mask_sb[:], in_=masks.ap())
        nc.vector.memset(vpad[:], 0.0)
        nc.vector.memset(vpad[:, S:S + 1], 1.0)
        nc.vector.memset(v3[:, :, :, S:S + 1], 1.0)

        # ---- V projection, all heads: V[t, s] (+ ones col) ----
        for tt in range(TT):
            pvv = pv_pool.tile([128, 4 * S], fp32, tag="pv")
            for et in range(ET):
                nc.tensor.matmul(pvv[:],
                                 xt[:, et, tt * 128:(tt + 1) * 128].bitcast(fp32r),
                                 wv_sb[:, et, :].bitcast(fp32r),
                                 start=(et == 0), stop=(et == ET - 1))
            nc.vector.tensor_copy(
                out=v3[:, tt, :, 0:S],
                in_=pvv[:, 0:NH * S].rearrange("p (g s) -> p g s", g=NH))

        # ---- per-head: Q.T/K.T projection, then attention ----
        for g in range(NH):
            qt = qk_pool.tile([64, T], fp32, tag="qt")
            kt = qk_pool.tile([64, T], fp32, tag="kt")
            for nch in range(NCH):
                sl = slice(nch * PROJN, (nch + 1) * PROJN)
                psq = st_pool.tile([64, PROJN], fp32, tag="st")
                for et in range(ET):
                    nc.tensor.matmul(psq[:], wqk_sb[:, et, g, 0:64].bitcast(fp32r),
                                     xt[:, et, sl].bitcast(fp32r),
                                     start=(et == 0), stop=(et == ET - 1))
                nc.scalar.copy(out=qt[:, sl].bitcast(fp32r), in_=psq[:])
                psk = st_pool.tile([64, PROJN], fp32, tag="st")
                for et in range(ET):
                    nc.tensor.matmul(psk[:], wqk_sb[:, et, g, 64:128].bitcast(fp32r),
                                     xt[:, et, sl].bitcast(fp32r),
                                     start=(et == 0), stop=(et == ET - 1))
                nc.vector.tensor_copy(out=kt[:, sl].bitcast(fp32r), in_=psk[:])

            for c in range(C):
                lo = 2 if c == 0 else 0        # first valid window key-tile
                hi = 4 if c == C - 1 else 6    # one past last valid
                stp = st_pool.tile([128, 6, WIN], fp32, tag="st")
                for w_i in range(lo, hi):
                    gk = (c - 1) * 2 + w_i
                    nc.tensor.matmul(stp[:, w_i, :],
                                     kt[:, gk * 128:(gk + 1) * 128].bitcast(fp32r),
                                     qt[:, c * WIN:(c + 1) * WIN].bitcast(fp32r),
                                     start=True, stop=True)
                ex = ex_pool.tile([128, 6, WIN], fp32)
                nc.scalar.activation(out=ex[:, lo:hi, :], in_=stp[:, lo:hi, :],
                                     func=Exp)
                pv = pv_pool.tile([128, 2, S + 1], fp32, tag="pv")
                for qh in range(2):
                    qsl = slice(qh * 128, (qh + 1) * 128)
                    for w_i in range(6):
                        if w_i < lo:
                            lhs = mask_sb[:, w_i, qsl]
                            rhs = vpad[:]
                        elif w_i >= hi:
                            lhs = mask_sb[:, 2 + (w_i - 4), qsl]
                            rhs = vpad[:]
                        else:
                            gk = (c - 1) * 2 + w_i
                            lhs = ex[:, w_i, qsl]
                            rhs = v3[:, gk, g, :]
                        nc.tensor.matmul(pv[:, qh, :], lhs, rhs,
                                         start=(w_i == 0), stop=(w_i == 5))
                rc = sm_pool.tile([128, 2, 1], fp32)
                nc.vector.reciprocal(rc[:], pv[:, :, S:S + 1])
                ob = o_pool.tile([128, 2, S], fp32)
                for qh in range(2):
                    nc.vector.tensor_scalar_mul(ob[:, qh, :], pv[:, qh, 0:S],
                                                rc[:, qh, 0:1])
                    nc.sync.dma_start(
                        out=out.ap()[c * WIN + qh * 128:c * WIN + (qh + 1) * 128,
                                     g * S:(g + 1) * S],
                        in_=ob[:, qh, :])

    with TileContext(nc) as tc:
        with ExitStack() as ctx:
            if loop_n is None:
                emit(tc, ctx)
            else:
                with tc.For_i(0, loop_n, 1):
                    emit(tc, ctx)
    nc.compile()
    return nc


def _make_masks():
    """0/1 multiplicative masks for the zero-padded prev/next blocks, in
    expS.T layout [key_within_tile, q]. Slots 0,1: chunk-0 prev tiles;
    slots 2,3: chunk-15 next tiles."""
    m = np.ones((128, 4, WIN), dtype=np.float32)
    p = np.arange(128)[:, None]
    q = np.arange(WIN)[None, :]
    for kt in range(2):
        k = kt * 128 + p
        m[:, kt, :] = np.where(q < WIN - k, 0.0, 1.0)
    for et in range(2):
        kn = et * 128 + p
        m[:, 2 + et, :] = np.where(q >= (WIN - 1) - kn, 0.0, 1.0)
    return m


def _prep_inputs(x, wq, wk, wv):
    """Host-side shard prep: per-core input dicts."""
    masks = _make_masks()
    xTb = [np.ascontiguousarray(x[b].T) for b in range(B)]
    wqs = wq.astype(np.float32) * np.float32(1.0 / math.sqrt(S))
    in_maps = []
    for core in range(NCORES):
        b, grp = divmod(core, 4)
        h0 = grp * NH
        wqk_np = np.empty((E, NH, 128), dtype=np.float32)
        wv_np = np.zeros((E, 4 * S), dtype=np.float32)
        for g in range(NH):
            h = h0 + g
            wqk_np[:, g, 0:64] = wqs[h * S:(h + 1) * S, :].T
            wqk_np[:, g, 64:128] = wk[h * S:(h + 1) * S, :].T
            wv_np[:, g * S:(g + 1) * S] = wv[h * S:(h + 1) * S, :].T
        in_maps.append({"xT": xTb[b], "wqk": wqk_np, "wv": wv_np,
                        "masks": masks})
    return in_maps


class _Runner:
    """Compile once; execute many times via PJRT across the 8 cores."""

    def __init__(self, loop_n=None):
        import jax
        import concourse.mybir as mybir
        from concourse import bass2jax
        from jax.sharding import Mesh, PartitionSpec
        from jax.experimental.shard_map import shard_map

        self.jax = jax
        nc = _build_module(loop_n=loop_n)
        self.nc = nc
        bass2jax.install_neuronx_cc_hook()

        partition_name = (nc.partition_id_tensor.name
                          if nc.partition_id_tensor else None)
        in_names, out_names, out_avals = [], [], []
        for alloc in nc.m.functions[0].allocations:
            if not isinstance(alloc, mybir.MemoryLocationSet):
                continue
            name = alloc.memorylocations[0].name
            if alloc.kind == "ExternalInput":
                if name != partition_name:
                    in_names.append(name)
            elif alloc.kind == "ExternalOutput":
                out_names.append(name)
                out_avals.append(jax.core.ShapedArray(
                    tuple(alloc.tensor_shape), mybir.dt.np(alloc.dtype)))
        self.in_names = in_names
        self.out_names = out_names
        n_params = len(in_names)
        n_outs = len(out_names)
        self.out_avals = out_avals
        in_names_all = list(in_names) + list(out_names)
        if partition_name:
            in_names_all.append(partition_name)

        def _body(*args):
            operands = list(args)
            if partition_name is not None:
                operands.append(bass2jax.partition_id_tensor())
            outs = bass2jax._bass_exec_p.bind(
                *operands, out_avals=tuple(out_avals),
                in_names=tuple(in_names_all), out_names=tuple(out_names),
                lowering_input_output_aliases=(),
                sim_require_finite=True, sim_require_nnan=True, nc=nc)
            return tuple(outs)

        devices = jax.devices()[:NCORES]
        mesh = Mesh(np.asarray(devices), ("core",))
        self._fn = jax.jit(
            shard_map(_body, mesh=mesh,
                      in_specs=(PartitionSpec("core"),) * (n_params + n_outs),
                      out_specs=(PartitionSpec("core"),) * n_outs,
                      check_rep=False),
            keep_unused=True)

    def put_args(self, in_maps):
        concat_in = [np.concatenate([m[nm] for m in in_maps], axis=0)
                     for nm in self.in_names]
        concat_zero = [np.zeros((NCORES * a.shape[0], *a.shape[1:]), a.dtype)
                       for a in self.out_avals]
        return [self.jax.device_put(a) for a in concat_in + concat_zero]

    def run(self, args):
        res = self.jax.block_until_ready(self._fn(*args))
        return [np.asarray(r) for r in res]


_RUNNER = None


def kernel(x, wq, wk, wv):
    global _RUNNER
    x = np.asarray(x, dtype=np.float32)
    wq = np.asarray(wq, dtype=np.float32)
    wk = np.asarray(wk, dtype=np.float32)
    wv = np.asarray(wv, dtype=np.float32)
    if _RUNNER is None:
        _RUNNER = _Runner()
    in_maps = _prep_inputs(x, wq, wk, wv)
    args = _RUNNER.put_args(in_maps)
    outs = _RUNNER.run(args)
    o = outs[0].reshape(NCORES, T, NH * S)
    full = np.empty((B, T, E), dtype=np.float32)
    for core in range(NCORES):
        b, grp = divmod(core, 4)
        full[b, :, grp * NH * S:(grp + 1) * NH * S] = o[core]
    return full


# revision 15
# speedup vs baseline: 4.7086x; 1.2405x over previous
"""Longformer-style sliding-chunk self-attention for Trainium2 (Bass/Tile).

Problem: B=2, T=4096, E=768, H=12 heads (head dim 64), window chunk W=256.
  q = (x @ wq.T)/8, k = x @ wk.T, v = x @ wv.T  (per head)
  scores: each chunk of 256 queries attends to [prev, cur, next] chunks
  (3*256 = 768 keys, zero-padded at sequence ends, with triangular masks on
  the pad blocks), softmax over the 768 window, then probs @ V.

Sharding: 8 cores = 2 batches x 4 head-groups of 3 heads. Each core gets
x[b].T (pre-transposed on host), per-head weight slices (transposed on
host, with the 1/8 query scale folded into wq), and produces
out[b, :, g*192:(g+1)*192].

Per-core kernel (all layouts chosen so no on-chip transposes are needed):
  - Q.T, K.T computed in [head_dim, T] layout (PSUM out of matmuls with
    weight slices as the stationary operand, x.T streaming).
  - V computed in natural [T, head_dim] layout (x.T tiles stationary,
    wv.T streaming), stored with a ones-column appended (V_aug) so the
    P@V matmul also produces the softmax denominator for free.
  - scores are computed TRANSPOSED: S.T[key, q] = K.T_tile.T @ Q.T_chunk,
    softmax uses exp WITHOUT max subtraction (scores ~ N(0,1), max < ~7,
    exp is safe in fp32) so no partition-dim reduction is ever needed.
  - P@V: out[q, s] = expS.T_tile.T @ V_aug accumulated over the 6 key
    tiles of the window; column 64 is the denominator; divide via
    reciprocal + tensor_scalar multiply.
  - boundary chunks: the zero-padded prev/next blocks have score 0, so
    exp(0)*mask = mask; the 0/1 mask tiles (precomputed on host) are used
    directly as the "expS" stationary operand with a zeros+ones V_aug pad
    tile, which also fixes the denominator. No masking work in the kernel.
"""

import math

import numpy as np

B, T, E, H, WIN = 2, 4096, 768, 12, 256
S = 64            # head dim
NH = 3            # heads per core
ET = E // 128     # 6 e-tiles
TT = T // 128     # 32 t-tiles
C = T // WIN      # 16 chunks
NCORES = 8
NCH = 8           # 512-wide column chunks for the projections
PROJN = T // NCH  # 512


def _build_module(loop_n=None, parts=("load", "vproj", "qkproj", "attn")):
    """Build + compile the per-core Bass module. Same program on all cores.

    parts: ablation control for timing experiments (kernel() always uses all).
    """
    from contextlib import ExitStack

    import concourse.mybir as mybir
    from concourse import bacc
    from concourse.tile import TileContext

    fp32 = mybir.dt.float32
    fp32r = mybir.dt.float32r
    Exp = mybir.ActivationFunctionType.Exp

    nc = bacc.Bacc("TRN2", target_bir_lowering=False, debug=False,
                   num_devices=NCORES)
    xT = nc.dram_tensor("xT", [E, T], fp32, kind="ExternalInput")
    wqk = nc.dram_tensor("wqk", [E, NH, 128], fp32, kind="ExternalInput")
    # wv is padded to 4*S=256 columns (last 64 zero) so the V projection's
    # moving dim is 256, which lets float32r run at 1 cycle/row.
    wv = nc.dram_tensor("wv", [E, 4 * S], fp32, kind="ExternalInput")
    masks = nc.dram_tensor("masks", [128, 4, WIN], fp32, kind="ExternalInput")
    out = nc.dram_tensor("out", [T, NH * S], fp32, kind="ExternalOutput")

    def emit(tc, ctx):
        del_unused = None
        singles = ctx.enter_context(tc.tile_pool(name="singles", bufs=1))
        qk_pool = ctx.enter_context(tc.tile_pool(name="qk", bufs=1))
        st_pool = ctx.enter_context(tc.tile_pool(name="st", bufs=2, space="PSUM"))
        pv_pool = ctx.enter_context(tc.tile_pool(name="pv", bufs=2, space="PSUM"))
        ex_pool = ctx.enter_context(tc.tile_pool(name="ex", bufs=2))
        o_pool = ctx.enter_context(tc.tile_pool(name="o", bufs=4))
        sm_pool = ctx.enter_context(tc.tile_pool(name="sm", bufs=4))

        # ---- persistent SBUF tensors ----
        xt = singles.tile([128, ET, T], fp32)            # x[b].T   96KB/part
        wqk_sb = singles.tile([128, ET, NH, 128], fp32)  # 9KB/part
        wv_sb = singles.tile([128, ET, 4 * S], fp32)     # 6KB/part
        mask_sb = singles.tile([128, 4, WIN], fp32)      # 4KB/part
        v3 = singles.tile([128, TT, NH, S + 1], fp32)    # V_aug  24.4KB/part
        vpad = singles.tile([128, S + 1], fp32)

        # ---- input loads ----
        # xt/wqk/wv feed float32r matmuls: the BIR verifier requires their
        # producers to emit float32r, so the loads are bitcast on both sides
        # (same 4-byte values; the PE does the hi/lo bf16 split at load).
        xT_r = xT.ap().bitcast(fp32r).rearrange("(a p) t -> a p t", p=128)
        if "load" in parts:
            for tq in range(4):
                for et in range(ET):
                    nc.sync.dma_start(out=xt[:, et, tq * 1024:(tq + 1) * 1024].bitcast(fp32r),
                                      in_=xT_r[et][:, tq * 1024:(tq + 1) * 1024])
        nc.sync.dma_start(out=wqk_sb[:].bitcast(fp32r),
                          in_=wqk.ap().bitcast(fp32r).rearrange("(a p) g m -> p a g m", p=128))
        nc.sync.dma_start(out=wv_sb[:].bitcast(fp32r),
                          in_=wv.ap().bitcast(fp32r).rearrange("(a p) m -> p a m", p=128))
        nc.sync.dma_start(out=mask_sb[:], in_=masks.ap())
        nc.vector.memset(vpad[:], 0.0)
        nc.vector.memset(vpad[:, S:S + 1], 1.0)
        nc.vector.memset(v3[:, :, :, S:S + 1], 1.0)

        # ---- V projection, all heads: V[t, s] (+ ones col) ----
        for tt in range(TT if "vproj" in parts else 0):
            pvv = pv_pool.tile([128, 4 * S], fp32, tag="pv")
            for et in range(ET):
                nc.tensor.matmul(pvv[:],
                                 xt[:, et, tt * 128:(tt + 1) * 128].bitcast(fp32r),
                                 wv_sb[:, et, :].bitcast(fp32r),
                                 start=(et == 0), stop=(et == ET - 1))
            nc.vector.tensor_copy(
                out=v3[:, tt, :, 0:S],
                in_=pvv[:, 0:NH * S].rearrange("p (g s) -> p g s", g=NH))

        # ---- per-head: Q.T/K.T projection, then attention ----
        for g in range(NH):
            qt = qk_pool.tile([64, T], fp32, tag="qt")
            kt = qk_pool.tile([64, T], fp32, tag="kt")
            if g == 0 and "attn" in parts and "qkproj" not in parts:
                nc.vector.memset(qt[:].bitcast(fp32r), 0.01)
                nc.vector.memset(kt[:].bitcast(fp32r), 0.01)
            for nch in range(NCH if "qkproj" in parts else 0):
                sl = slice(nch * PROJN, (nch + 1) * PROJN)
                psq = st_pool.tile([64, PROJN], fp32, tag="st")
                for et in range(ET):
                    nc.tensor.matmul(psq[:], wqk_sb[:, et, g, 0:64].bitcast(fp32r),
                                     xt[:, et, sl].bitcast(fp32r),
                                     start=(et == 0), stop=(et == ET - 1))
                nc.scalar.copy(out=qt[:, sl].bitcast(fp32r), in_=psq[:])
                psk = st_pool.tile([64, PROJN], fp32, tag="st")
                for et in range(ET):
                    nc.tensor.matmul(psk[:], wqk_sb[:, et, g, 64:128].bitcast(fp32r),
                                     xt[:, et, sl].bitcast(fp32r),
                                     start=(et == 0), stop=(et == ET - 1))
                nc.vector.tensor_copy(out=kt[:, sl].bitcast(fp32r), in_=psk[:])

            for c in range(C if "attn" in parts else 0):
                lo = 2 if c == 0 else 0        # first valid window key-tile
                hi = 4 if c == C - 1 else 6    # one past last valid
                stp = st_pool.tile([128, 6, WIN], fp32, tag="st")
                for w_i in range(lo, hi):
                    gk = (c - 1) * 2 + w_i
                    nc.tensor.matmul(stp[:, w_i, :],
                                     kt[:, gk * 128:(gk + 1) * 128].bitcast(fp32r),
                                     qt[:, c * WIN:(c + 1) * WIN].bitcast(fp32r),
                                     start=True, stop=True)
                ex = ex_pool.tile([128, 6, WIN], fp32)
                nc.scalar.activation(out=ex[:, lo:hi, :], in_=stp[:, lo:hi, :],
                                     func=Exp)
                pv = pv_pool.tile([128, 2, S + 1], fp32, tag="pv")
                for qh in range(2):
                    qsl = slice(qh * 128, (qh + 1) * 128)
                    for w_i in range(6):
                        if w_i < lo:
                            lhs = mask_sb[:, w_i, qsl]
                            rhs = vpad[:]
                        elif w_i >= hi:
                            lhs = mask_sb[:, 2 + (w_i - 4), qsl]
                            rhs = vpad[:]
                        else:
                            gk = (c - 1) * 2 + w_i
                            lhs = ex[:, w_i, qsl]
                            rhs = v3[:, gk, g, :]
                        nc.tensor.matmul(pv[:, qh, :], lhs, rhs,
                                         start=(w_i == 0), stop=(w_i == 5))
                rc = sm_pool.tile([128, 2, 1], fp32)
                nc.vector.reciprocal(rc[:], pv[:, :, S:S + 1])
                ob = o_pool.tile([128, 2, S], fp32)
                for qh in range(2):
                    nc.vector.tensor_scalar_mul(ob[:, qh, :], pv[:, qh, 0:S],
                                                rc[:, qh, 0:1])
                nc.sync.dma_start(
                    out=out.ap()[c * WIN:(c + 1) * WIN, g * S:(g + 1) * S]
                        .rearrange("(q2 p) s -> p q2 s", p=128),
                    in_=ob[:])

    with TileContext(nc) as tc:
        with ExitStack() as ctx:
            if loop_n is None:
                emit(tc, ctx)
            else:
                with tc.For_i(0, loop_n, 1):
                    emit(tc, ctx)
    nc.compile()
    return nc


def _make_masks():
    """0/1 multiplicative masks for the zero-padded prev/next blocks, in
    expS.T layout [key_within_tile, q]. Slots 0,1: chunk-0 prev tiles;
    slots 2,3: chunk-15 next tiles."""
    m = np.ones((128, 4, WIN), dtype=np.float32)
    p = np.arange(128)[:, None]
    q = np.arange(WIN)[None, :]
    for kt in range(2):
        k = kt * 128 + p
        m[:, kt, :] = np.where(q < WIN - k, 0.0, 1.0)
    for et in range(2):
        kn = et * 128 + p
        m[:, 2 + et, :] = np.where(q >= (WIN - 1) - kn, 0.0, 1.0)
    return m


def _prep_inputs(x, wq, wk, wv):
    """Host-side shard prep: per-core input dicts."""
    masks = _make_masks()
    xTb = [np.ascontiguousarray(x[b].T) for b in range(B)]
    wqs = wq.astype(np.float32) * np.float32(1.0 / math.sqrt(S))
    in_maps = []
    for core in range(NCORES):
        b, grp = divmod(core, 4)
        h0 = grp * NH
        wqk_np = np.empty((E, NH, 128), dtype=np.float32)
        wv_np = np.zeros((E, 4 * S), dtype=np.float32)
        for g in range(NH):
            h = h0 + g
            wqk_np[:, g, 0:64] = wqs[h * S:(h + 1) * S, :].T
            wqk_np[:, g, 64:128] = wk[h * S:(h + 1) * S, :].T
            wv_np[:, g * S:(g + 1) * S] = wv[h * S:(h + 1) * S, :].T
        in_maps.append({"xT": xTb[b], "wqk": wqk_np, "wv": wv_np,
                        "masks": masks})
    return in_maps


class _Runner:
    """Compile once; execute many times via PJRT across the 8 cores."""

    def __init__(self, loop_n=None):
        import jax
        import concourse.mybir as mybir
        from concourse import bass2jax
        from jax.sharding import Mesh, PartitionSpec
        from jax.experimental.shard_map import shard_map

        self.jax = jax
        nc = _build_module(loop_n=loop_n)
        self.nc = nc
        bass2jax.install_neuronx_cc_hook()

        partition_name = (nc.partition_id_tensor.name
                          if nc.partition_id_tensor else None)
        in_names, out_names, out_avals = [], [], []
        for alloc in nc.m.functions[0].allocations:
            if not isinstance(alloc, mybir.MemoryLocationSet):
                continue
            name = alloc.memorylocations[0].name
            if alloc.kind == "ExternalInput":
                if name != partition_name:
                    in_names.append(name)
            elif alloc.kind == "ExternalOutput":
                out_names.append(name)
                out_avals.append(jax.core.ShapedArray(
                    tuple(alloc.tensor_shape), mybir.dt.np(alloc.dtype)))
        self.in_names = in_names
        self.out_names = out_names
        n_params = len(in_names)
        n_outs = len(out_names)
        self.out_avals = out_avals
        in_names_all = list(in_names) + list(out_names)
        if partition_name:
            in_names_all.append(partition_name)

        def _body(*args):
            operands = list(args)
            if partition_name is not None:
                operands.append(bass2jax.partition_id_tensor())
            outs = bass2jax._bass_exec_p.bind(
                *operands, out_avals=tuple(out_avals),
                in_names=tuple(in_names_all), out_names=tuple(out_names),
                lowering_input_output_aliases=(),
                sim_require_finite=True, sim_require_nnan=True, nc=nc)
            return tuple(outs)

        devices = jax.devices()[:NCORES]
        mesh = Mesh(np.asarray(devices), ("core",))
        self._fn = jax.jit(
            shard_map(_body, mesh=mesh,
                      in_specs=(PartitionSpec("core"),) * (n_params + n_outs),
                      out_specs=(PartitionSpec("core"),) * n_outs,
                      check_rep=False),
            keep_unused=True)

    def put_args(self, in_maps):
        concat_in = [np.concatenate([m[nm] for m in in_maps], axis=0)
                     for nm in self.in_names]
        concat_zero = [np.zeros((NCORES * a.shape[0], *a.shape[1:]), a.dtype)
                       for a in self.out_avals]
        return [self.jax.device_put(a) for a in concat_in + concat_zero]

    def run(self, args):
        res = self.jax.block_until_ready(self._fn(*args))
        return [np.asarray(r) for r in res]


_RUNNER = None


def kernel(x, wq, wk, wv):
    global _RUNNER
    x = np.asarray(x, dtype=np.float32)
    wq = np.asarray(wq, dtype=np.float32)
    wk = np.asarray(wk, dtype=np.float32)
    wv = np.asarray(wv, dtype=np.float32)
    if _RUNNER is None:
        _RUNNER = _Runner()
    in_maps = _prep_inputs(x, wq, wk, wv)
    args = _RUNNER.put_args(in_maps)
    outs = _RUNNER.run(args)
    o = outs[0].reshape(NCORES, T, NH * S)
    full = np.empty((B, T, E), dtype=np.float32)
    for core in range(NCORES):
        b, grp = divmod(core, 4)
        full[b, :, grp * NH * S:(grp + 1) * NH * S] = o[core]
    return full


# revision 16
# speedup vs baseline: 5.6276x; 1.1952x over previous
"""Longformer-style sliding-chunk self-attention for Trainium2 (Bass/Tile).

Problem: B=2, T=4096, E=768, H=12 heads (head dim 64), window chunk W=256.
  q = (x @ wq.T)/8, k = x @ wk.T, v = x @ wv.T  (per head)
  scores: each chunk of 256 queries attends to [prev, cur, next] chunks
  (3*256 = 768 keys, zero-padded at sequence ends, with triangular masks on
  the pad blocks), softmax over the 768 window, then probs @ V.

Sharding: 8 cores = 2 batches x 4 head-groups of 3 heads. Each core gets
x[b].T (pre-transposed on host), per-head weight slices (transposed on
host, with the 1/8 query scale folded into wq), and produces
out[b, :, g*192:(g+1)*192].

Per-core kernel (all layouts chosen so no on-chip transposes are needed):
  - Q.T, K.T computed in [head_dim, T] layout (PSUM out of matmuls with
    weight slices as the stationary operand, x.T streaming).
  - V computed in natural [T, head_dim] layout (x.T tiles stationary,
    wv.T streaming), stored with a ones-column appended (V_aug) so the
    P@V matmul also produces the softmax denominator for free.
  - scores are computed TRANSPOSED: S.T[key, q] = K.T_tile.T @ Q.T_chunk,
    softmax uses exp WITHOUT max subtraction (scores ~ N(0,1), max < ~7,
    exp is safe in fp32) so no partition-dim reduction is ever needed.
  - P@V: out[q, s] = expS.T_tile.T @ V_aug accumulated over the 6 key
    tiles of the window; column 64 is the denominator; divide via
    reciprocal + tensor_scalar multiply.
  - boundary chunks: the zero-padded prev/next blocks have score 0, so
    exp(0)*mask = mask; the 0/1 mask tiles (precomputed on host) are used
    directly as the "expS" stationary operand with a zeros+ones V_aug pad
    tile, which also fixes the denominator. No masking work in the kernel.
"""

import math

import numpy as np

B, T, E, H, WIN = 2, 4096, 768, 12, 256
S = 64            # head dim
NH = 3            # heads per core
ET = E // 128     # 6 e-tiles
TT = T // 128     # 32 t-tiles
C = T // WIN      # 16 chunks
NCORES = 8
NCH = 8           # 512-wide column chunks for the projections
PROJN = T // NCH  # 512


def _build_module(loop_n=None, parts=("load", "vproj", "qkproj", "attn")):
    """Build + compile the per-core Bass module. Same program on all cores.

    parts: ablation control for timing experiments (kernel() always uses all).
    """
    from contextlib import ExitStack

    import concourse.mybir as mybir
    from concourse import bacc
    from concourse.tile import TileContext

    fp32 = mybir.dt.float32
    fp32r = mybir.dt.float32r
    Exp = mybir.ActivationFunctionType.Exp

    nc = bacc.Bacc("TRN2", target_bir_lowering=False, debug=False,
                   num_devices=NCORES)
    xT = nc.dram_tensor("xT", [E, T], fp32, kind="ExternalInput")
    wqk = nc.dram_tensor("wqk", [E, NH, 128], fp32, kind="ExternalInput")
    # wv is padded to 4*S=256 columns (last 64 zero) so the V projection's
    # moving dim is 256, which lets float32r run at 1 cycle/row.
    wv = nc.dram_tensor("wv", [E, 4 * S], fp32, kind="ExternalInput")
    masks = nc.dram_tensor("masks", [128, 4, WIN], fp32, kind="ExternalInput")
    out = nc.dram_tensor("out", [T, NH * S], fp32, kind="ExternalOutput")

    def emit(tc, ctx):
        del_unused = None
        singles = ctx.enter_context(tc.tile_pool(name="singles", bufs=1))
        qk_pool = ctx.enter_context(tc.tile_pool(name="qk", bufs=1))
        st_pool = ctx.enter_context(tc.tile_pool(name="st", bufs=2, space="PSUM"))
        pv_pool = ctx.enter_context(tc.tile_pool(name="pv", bufs=2, space="PSUM"))
        ex_pool = ctx.enter_context(tc.tile_pool(name="ex", bufs=2))
        o_pool = ctx.enter_context(tc.tile_pool(name="o", bufs=4))
        sm_pool = ctx.enter_context(tc.tile_pool(name="sm", bufs=4))

        # ---- persistent SBUF tensors ----
        xt = singles.tile([128, ET, T], fp32)            # x[b].T   96KB/part
        wqk_sb = singles.tile([128, ET, NH, 128], fp32)  # 9KB/part
        wv_sb = singles.tile([128, ET, 4 * S], fp32)     # 6KB/part
        mask_sb = singles.tile([128, 4, WIN], fp32)      # 4KB/part
        v3 = singles.tile([128, TT, NH, S + 1], fp32)    # V_aug  24.4KB/part
        vpad = singles.tile([128, S + 1], fp32)

        # ---- input loads ----
        # xt/wqk/wv feed float32r matmuls: the BIR verifier requires their
        # producers to emit float32r, so the loads are bitcast on both sides
        # (same 4-byte values; the PE does the hi/lo bf16 split at load).
        xT_r = xT.ap().bitcast(fp32r).rearrange("(a p) t -> a p t", p=128)
        if "load" in parts:
            for tq in range(4):
                for et in range(ET):
                    nc.sync.dma_start(out=xt[:, et, tq * 1024:(tq + 1) * 1024].bitcast(fp32r),
                                      in_=xT_r[et][:, tq * 1024:(tq + 1) * 1024])
        nc.sync.dma_start(out=wqk_sb[:].bitcast(fp32r),
                          in_=wqk.ap().bitcast(fp32r).rearrange("(a p) g m -> p a g m", p=128))
        nc.sync.dma_start(out=wv_sb[:].bitcast(fp32r),
                          in_=wv.ap().bitcast(fp32r).rearrange("(a p) m -> p a m", p=128))
        nc.sync.dma_start(out=mask_sb[:], in_=masks.ap())
        nc.vector.memset(vpad[:], 0.0)
        nc.vector.memset(vpad[:, S:S + 1], 1.0)
        nc.vector.memset(v3[:, :, :, S:S + 1], 1.0)

        # ---- V projection, all heads: V[t, s] (+ ones col) ----
        for tt in range(TT if "vproj" in parts else 0):
            pvv = pv_pool.tile([128, 4 * S], fp32, tag="pv")
            for et in range(ET):
                nc.tensor.matmul(pvv[:],
                                 xt[:, et, tt * 128:(tt + 1) * 128].bitcast(fp32r),
                                 wv_sb[:, et, :].bitcast(fp32r),
                                 start=(et == 0), stop=(et == ET - 1))
            nc.vector.tensor_copy(
                out=v3[:, tt, :, 0:S],
                in_=pvv[:, 0:NH * S].rearrange("p (g s) -> p g s", g=NH))

        # ---- per-head: Q.T/K.T projection, then attention ----
        for g in range(NH):
            qt = qk_pool.tile([64, T], fp32, tag="qt")
            kt = qk_pool.tile([64, T], fp32, tag="kt")
            if g == 0 and "attn" in parts and "qkproj" not in parts:
                nc.vector.memset(qt[:].bitcast(fp32r), 0.01)
                nc.vector.memset(kt[:].bitcast(fp32r), 0.01)
            for nch in range(NCH if "qkproj" in parts else 0):
                sl = slice(nch * PROJN, (nch + 1) * PROJN)
                psq = st_pool.tile([64, PROJN], fp32, tag="st")
                for et in range(ET):
                    nc.tensor.matmul(psq[:], wqk_sb[:, et, g, 0:64].bitcast(fp32r),
                                     xt[:, et, sl].bitcast(fp32r),
                                     start=(et == 0), stop=(et == ET - 1))
                nc.scalar.copy(out=qt[:, sl].bitcast(fp32r), in_=psq[:])
                psk = st_pool.tile([64, PROJN], fp32, tag="st")
                for et in range(ET):
                    nc.tensor.matmul(psk[:], wqk_sb[:, et, g, 64:128].bitcast(fp32r),
                                     xt[:, et, sl].bitcast(fp32r),
                                     start=(et == 0), stop=(et == ET - 1))
                nc.vector.tensor_copy(out=kt[:, sl].bitcast(fp32r), in_=psk[:])

            # Software-pipelined attention: the PE is in-order, so PV(c) right
            # after QK(c) would stall on exp(c). Emit QK(c+1) between exp(c)
            # and PV(c) so the PE always has independent matmuls in hand.
            def emit_qk(c):
                lo = 2 if c == 0 else 0        # first valid window key-tile
                hi = 4 if c == C - 1 else 6    # one past last valid
                stp = st_pool.tile([128, 6, WIN], fp32, tag="st")
                for w_i in range(lo, hi):
                    gk = (c - 1) * 2 + w_i
                    nc.tensor.matmul(stp[:, w_i, :],
                                     kt[:, gk * 128:(gk + 1) * 128].bitcast(fp32r),
                                     qt[:, c * WIN:(c + 1) * WIN].bitcast(fp32r),
                                     start=True, stop=True)
                ex = ex_pool.tile([128, 6, WIN], fp32)
                nc.scalar.activation(out=ex[:, lo:hi, :], in_=stp[:, lo:hi, :],
                                     func=Exp)
                return (c, lo, hi, ex)

            def emit_pv(state):
                c, lo, hi, ex = state
                pv = pv_pool.tile([128, 2, S + 1], fp32, tag="pv")
                for qh in range(2):
                    qsl = slice(qh * 128, (qh + 1) * 128)
                    for w_i in range(6):
                        if w_i < lo:
                            lhs = mask_sb[:, w_i, qsl]
                            rhs = vpad[:]
                        elif w_i >= hi:
                            lhs = mask_sb[:, 2 + (w_i - 4), qsl]
                            rhs = vpad[:]
                        else:
                            gk = (c - 1) * 2 + w_i
                            lhs = ex[:, w_i, qsl]
                            rhs = v3[:, gk, g, :]
                        nc.tensor.matmul(pv[:, qh, :], lhs, rhs,
                                         start=(w_i == 0), stop=(w_i == 5))
                rc = sm_pool.tile([128, 2, 1], fp32)
                nc.vector.reciprocal(rc[:], pv[:, :, S:S + 1])
                ob = o_pool.tile([128, 2, S], fp32)
                for qh in range(2):
                    nc.vector.tensor_scalar_mul(ob[:, qh, :], pv[:, qh, 0:S],
                                                rc[:, qh, 0:1])
                nc.sync.dma_start(
                    out=out.ap()[c * WIN:(c + 1) * WIN, g * S:(g + 1) * S]
                        .rearrange("(q2 p) s -> p q2 s", p=128),
                    in_=ob[:])

            if "attn" in parts:
                pending = emit_qk(0)
                for c in range(1, C):
                    nxt = emit_qk(c)
                    emit_pv(pending)
                    pending = nxt
                emit_pv(pending)

    with TileContext(nc) as tc:
        with ExitStack() as ctx:
            if loop_n is None:
                emit(tc, ctx)
            else:
                with tc.For_i(0, loop_n, 1):
                    emit(tc, ctx)
    nc.compile()
    return nc


def _make_masks():
    """0/1 multiplicative masks for the zero-padded prev/next blocks, in
    expS.T layout [key_within_tile, q]. Slots 0,1: chunk-0 prev tiles;
    slots 2,3: chunk-15 next tiles."""
    m = np.ones((128, 4, WIN), dtype=np.float32)
    p = np.arange(128)[:, None]
    q = np.arange(WIN)[None, :]
    for kt in range(2):
        k = kt * 128 + p
        m[:, kt, :] = np.where(q < WIN - k, 0.0, 1.0)
    for et in range(2):
        kn = et * 128 + p
        m[:, 2 + et, :] = np.where(q >= (WIN - 1) - kn, 0.0, 1.0)
    return m


def _prep_inputs(x, wq, wk, wv):
    """Host-side shard prep: per-core input dicts."""
    masks = _make_masks()
    xTb = [np.ascontiguousarray(x[b].T) for b in range(B)]
    wqs = wq.astype(np.float32) * np.float32(1.0 / math.sqrt(S))
    in_maps = []
    for core in range(NCORES):
        b, grp = divmod(core, 4)
        h0 = grp * NH
        wqk_np = np.empty((E, NH, 128), dtype=np.float32)
        wv_np = np.zeros((E, 4 * S), dtype=np.float32)
        for g in range(NH):
            h = h0 + g
            wqk_np[:, g, 0:64] = wqs[h * S:(h + 1) * S, :].T
            wqk_np[:, g, 64:128] = wk[h * S:(h + 1) * S, :].T
            wv_np[:, g * S:(g + 1) * S] = wv[h * S:(h + 1) * S, :].T
        in_maps.append({"xT": xTb[b], "wqk": wqk_np, "wv": wv_np,
                        "masks": masks})
    return in_maps


class _Runner:
    """Compile once; execute many times via PJRT across the 8 cores."""

    def __init__(self, loop_n=None):
        import jax
        import concourse.mybir as mybir
        from concourse import bass2jax
        from jax.sharding import Mesh, PartitionSpec
        from jax.experimental.shard_map import shard_map

        self.jax = jax
        nc = _build_module(loop_n=loop_n)
        self.nc = nc
        bass2jax.install_neuronx_cc_hook()

        partition_name = (nc.partition_id_tensor.name
                          if nc.partition_id_tensor else None)
        in_names, out_names, out_avals = [], [], []
        for alloc in nc.m.functions[0].allocations:
            if not isinstance(alloc, mybir.MemoryLocationSet):
                continue
            name = alloc.memorylocations[0].name
            if alloc.kind == "ExternalInput":
                if name != partition_name:
                    in_names.append(name)
            elif alloc.kind == "ExternalOutput":
                out_names.append(name)
                out_avals.append(jax.core.ShapedArray(
                    tuple(alloc.tensor_shape), mybir.dt.np(alloc.dtype)))
        self.in_names = in_names
        self.out_names = out_names
        n_params = len(in_names)
        n_outs = len(out_names)
        self.out_avals = out_avals
        in_names_all = list(in_names) + list(out_names)
        if partition_name:
            in_names_all.append(partition_name)

        def _body(*args):
            operands = list(args)
            if partition_name is not None:
                operands.append(bass2jax.partition_id_tensor())
            outs = bass2jax._bass_exec_p.bind(
                *operands, out_avals=tuple(out_avals),
                in_names=tuple(in_names_all), out_names=tuple(out_names),
                lowering_input_output_aliases=(),
                sim_require_finite=True, sim_require_nnan=True, nc=nc)
            return tuple(outs)

        devices = jax.devices()[:NCORES]
        mesh = Mesh(np.asarray(devices), ("core",))
        self._fn = jax.jit(
            shard_map(_body, mesh=mesh,
                      in_specs=(PartitionSpec("core"),) * (n_params + n_outs),
                      out_specs=(PartitionSpec("core"),) * n_outs,
                      check_rep=False),
            keep_unused=True)

    def put_args(self, in_maps):
        concat_in = [np.concatenate([m[nm] for m in in_maps], axis=0)
                     for nm in self.in_names]
        concat_zero = [np.zeros((NCORES * a.shape[0], *a.shape[1:]), a.dtype)
                       for a in self.out_avals]
        return [self.jax.device_put(a) for a in concat_in + concat_zero]

    def run(self, args):
        res = self.jax.block_until_ready(self._fn(*args))
        return [np.asarray(r) for r in res]


_RUNNER = None


def kernel(x, wq, wk, wv):
    global _RUNNER
    x = np.asarray(x, dtype=np.float32)
    wq = np.asarray(wq, dtype=np.float32)
    wk = np.asarray(wk, dtype=np.float32)
    wv = np.asarray(wv, dtype=np.float32)
    if _RUNNER is None:
        _RUNNER = _Runner()
    in_maps = _prep_inputs(x, wq, wk, wv)
    args = _RUNNER.put_args(in_maps)
    outs = _RUNNER.run(args)
    o = outs[0].reshape(NCORES, T, NH * S)
    full = np.empty((B, T, E), dtype=np.float32)
    for core in range(NCORES):
        b, grp = divmod(core, 4)
        full[b, :, grp * NH * S:(grp + 1) * NH * S] = o[core]
    return full


# revision 19
# speedup vs baseline: 8.7410x; 1.5532x over previous
"""Longformer-style sliding-chunk self-attention for Trainium2 (Bass/Tile).

Problem: B=2, T=4096, E=768, H=12 heads (head dim 64), window chunk W=256.
  q = (x @ wq.T)/8, k = x @ wk.T, v = x @ wv.T  (per head)
  scores: each chunk of 256 queries attends to [prev, cur, next] chunks
  (3*256 = 768 keys, zero-padded at sequence ends, with triangular masks on
  the pad blocks), softmax over the 768 window, then probs @ V.

Sharding: 8 cores = 2 batches x 4 head-groups of 3 heads. Each core gets
x[b].T (pre-transposed on host), per-head weight slices (transposed on
host, with the 1/8 query scale folded into wq), and produces
out[b, :, g*192:(g+1)*192].

Per-core kernel (all layouts chosen so no on-chip transposes are needed):
  - Q.T, K.T computed in [head_dim, T] layout (PSUM out of matmuls with
    weight slices as the stationary operand, x.T streaming).
  - V computed in natural [T, head_dim] layout (x.T tiles stationary,
    wv.T streaming), stored with a ones-column appended (V_aug) so the
    P@V matmul also produces the softmax denominator for free.
  - scores are computed TRANSPOSED: S.T[key, q] = K.T_tile.T @ Q.T_chunk,
    softmax uses exp WITHOUT max subtraction (scores ~ N(0,1), max < ~7,
    exp is safe in fp32) so no partition-dim reduction is ever needed.
  - P@V: out[q, s] = expS.T_tile.T @ V_aug accumulated over the 6 key
    tiles of the window; column 64 is the denominator; divide via
    reciprocal + tensor_scalar multiply.
  - boundary chunks: the zero-padded prev/next blocks have score 0, so
    exp(0)*mask = mask; the 0/1 mask tiles (precomputed on host) are used
    directly as the "expS" stationary operand with a zeros+ones V_aug pad
    tile, which also fixes the denominator. No masking work in the kernel.
"""

import math

import numpy as np

B, T, E, H, WIN = 2, 4096, 768, 12, 256
S = 64            # head dim
NH = 3            # heads per core
ET = E // 128     # 6 e-tiles
TT = T // 128     # 32 t-tiles
C = T // WIN      # 16 chunks
NCORES = 8
NCH = 8           # 512-wide column chunks for the projections
PROJN = T // NCH  # 512


def _build_module(loop_n=None, parts=("load", "vproj", "qkproj", "attn")):
    """Build + compile the per-core Bass module. Same program on all cores.

    parts: ablation control for timing experiments (kernel() always uses all).
    """
    from contextlib import ExitStack

    import concourse.mybir as mybir
    from concourse import bacc
    from concourse.tile import TileContext

    fp32 = mybir.dt.float32
    fp32r = mybir.dt.float32r
    Exp = mybir.ActivationFunctionType.Exp

    nc = bacc.Bacc("TRN2", target_bir_lowering=False, debug=False,
                   num_devices=NCORES)
    xT = nc.dram_tensor("xT", [E, T], fp32, kind="ExternalInput")
    wqk = nc.dram_tensor("wqk", [E, NH, 128], fp32, kind="ExternalInput")
    # wv is padded to 4*S=256 columns (last 64 zero) so the V projection's
    # moving dim is 256, which lets float32r run at 1 cycle/row.
    wv = nc.dram_tensor("wv", [E, 4 * S], fp32, kind="ExternalInput")
    masks = nc.dram_tensor("masks", [128, 4, WIN], fp32, kind="ExternalInput")
    ident = nc.dram_tensor("ident", [S + 1, S + 1], fp32, kind="ExternalInput")
    out = nc.dram_tensor("out", [T, NH * S], fp32, kind="ExternalOutput")

    def emit(tc, ctx):
        del_unused = None
        singles = ctx.enter_context(tc.tile_pool(name="singles", bufs=1))
        qk_pool = ctx.enter_context(tc.tile_pool(name="qk", bufs=1))
        st_pool = ctx.enter_context(tc.tile_pool(name="st", bufs=2, space="PSUM"))
        pv_pool = ctx.enter_context(tc.tile_pool(name="pv", bufs=2, space="PSUM"))
        ex_pool = ctx.enter_context(tc.tile_pool(name="ex", bufs=2))
        o_pool = ctx.enter_context(tc.tile_pool(name="o", bufs=4))
        cx_pool = ctx.enter_context(tc.tile_pool(name="cx", bufs=3))
        sm_pool = ctx.enter_context(tc.tile_pool(name="sm", bufs=4))

        # ---- persistent SBUF tensors ----
        xt = singles.tile([128, ET, T], fp32)            # x[b].T   96KB/part
        wqk_sb = singles.tile([128, ET, NH, 128], fp32)  # 9KB/part
        wv_sb = singles.tile([128, ET, 4 * S], fp32)     # 6KB/part
        mask_sb = singles.tile([128, 4, WIN], fp32)      # 4KB/part
        v3 = singles.tile([128, TT, NH, S + 1], fp32)    # V_aug  24.4KB/part
        vpad = singles.tile([128, S + 1], fp32)
        ident_sb = singles.tile([S + 1, S + 1], fp32)

        # ---- input loads ----
        # xt/wqk/wv feed float32r matmuls: the BIR verifier requires their
        # producers to emit float32r, so the loads are bitcast on both sides
        # (same 4-byte values; the PE does the hi/lo bf16 split at load).
        xT_r = xT.ap().bitcast(fp32r).rearrange("(a p) t -> a p t", p=128)
        if "load" in parts:
            for tq in range(4):
                for et in range(ET):
                    nc.sync.dma_start(out=xt[:, et, tq * 1024:(tq + 1) * 1024].bitcast(fp32r),
                                      in_=xT_r[et][:, tq * 1024:(tq + 1) * 1024])
        nc.sync.dma_start(out=wqk_sb[:].bitcast(fp32r),
                          in_=wqk.ap().bitcast(fp32r).rearrange("(a p) g m -> p a g m", p=128))
        nc.sync.dma_start(out=wv_sb[:].bitcast(fp32r),
                          in_=wv.ap().bitcast(fp32r).rearrange("(a p) m -> p a m", p=128))
        nc.sync.dma_start(out=mask_sb[:].bitcast(fp32r),
                          in_=masks.ap().bitcast(fp32r))
        nc.sync.dma_start(out=ident_sb[:], in_=ident.ap())
        # DVE memset cannot emit float32r; stage the constants in fp32 and
        # round through ACT copies (valid float32r producers).
        cst = singles.tile([128, S + 1 + TT * NH], fp32)
        nc.vector.memset(cst[:], 0.0)
        nc.vector.memset(cst[:, S:S + 1], 1.0)
        nc.vector.memset(cst[:, S + 1:], 1.0)
        nc.scalar.copy(out=vpad[:].bitcast(fp32r), in_=cst[:, 0:S + 1])
        nc.scalar.copy(
            out=v3[:, :, :, S:S + 1].bitcast(fp32r),
            in_=cst[:, S + 1:].rearrange("p (a g one) -> p a g one",
                                         a=TT, g=NH, one=1))

        # ---- V projection, all heads: V[t, s] (+ ones col) ----
        for tt in range(TT if "vproj" in parts else 0):
            pvv = pv_pool.tile([128, 4 * S], fp32, tag="ctx")
            for et in range(ET):
                nc.tensor.matmul(pvv[:],
                                 xt[:, et, tt * 128:(tt + 1) * 128].bitcast(fp32r),
                                 wv_sb[:, et, :].bitcast(fp32r),
                                 start=(et == 0), stop=(et == ET - 1))
            nc.vector.tensor_copy(
                out=v3[:, tt, :, 0:S].bitcast(fp32r),
                in_=pvv[:, 0:NH * S].rearrange("p (g s) -> p g s", g=NH))

        # ---- per-head: Q.T/K.T projection, then attention ----
        for g in range(NH):
            qt = qk_pool.tile([64, T], fp32, tag="qt")
            kt = qk_pool.tile([64, T], fp32, tag="kt")
            if g == 0 and "attn" in parts and "qkproj" not in parts:
                nc.vector.memset(qt[:], 0.01)
                nc.vector.memset(kt[:], 0.01)
            for nch in range(NCH if "qkproj" in parts else 0):
                sl = slice(nch * PROJN, (nch + 1) * PROJN)
                psq = pv_pool.tile([64, PROJN], fp32, tag="ctx")
                for et in range(ET):
                    nc.tensor.matmul(psq[:], wqk_sb[:, et, g, 0:64].bitcast(fp32r),
                                     xt[:, et, sl].bitcast(fp32r),
                                     start=(et == 0), stop=(et == ET - 1))
                nc.scalar.copy(out=qt[:, sl].bitcast(fp32r), in_=psq[:])
                psk = pv_pool.tile([64, PROJN], fp32, tag="ctx")
                for et in range(ET):
                    nc.tensor.matmul(psk[:], wqk_sb[:, et, g, 64:128].bitcast(fp32r),
                                     xt[:, et, sl].bitcast(fp32r),
                                     start=(et == 0), stop=(et == ET - 1))
                nc.vector.tensor_copy(out=kt[:, sl].bitcast(fp32r), in_=psk[:])

            # Software-pipelined attention: the PE is in-order, so PV(c) right
            # after QK(c) would stall on exp(c). Emit QK(c+1) between exp(c)
            # and PV(c) so the PE always has independent matmuls in hand.
            def emit_qk(c):
                lo = 2 if c == 0 else 0        # first valid window key-tile
                hi = 4 if c == C - 1 else 6    # one past last valid
                stp = st_pool.tile([128, 6, WIN], fp32, tag="st")
                for w_i in range(lo, hi):
                    gk = (c - 1) * 2 + w_i
                    nc.tensor.matmul(stp[:, w_i, :],
                                     kt[:, gk * 128:(gk + 1) * 128].bitcast(fp32r),
                                     qt[:, c * WIN:(c + 1) * WIN].bitcast(fp32r),
                                     start=True, stop=True)
                ex = ex_pool.tile([128, 6, WIN], fp32)
                nc.scalar.activation(out=ex[:, lo:hi, :].bitcast(fp32r),
                                     in_=stp[:, lo:hi, :], func=Exp)
                return (c, lo, hi, ex)

            def emit_pv(state):
                # ctx.T[s|denom, q] = sum_k V_aug[k, s] * expS.T[k, q]:
                # V_aug stationary (65 cols -> cheap weight load), expS.T
                # moving (256 -> full-rate fp32r streaming).
                c, lo, hi, ex = state
                ctxT = pv_pool.tile([S + 1, WIN], fp32, tag="ctx")
                for w_i in range(6):
                    if w_i < lo:
                        sta, mov = vpad[:], mask_sb[:, w_i, :]
                    elif w_i >= hi:
                        sta, mov = vpad[:], mask_sb[:, 2 + (w_i - 4), :]
                    else:
                        gk = (c - 1) * 2 + w_i
                        sta, mov = v3[:, gk, g, :], ex[:, w_i, :]
                    nc.tensor.matmul(ctxT[:], sta.bitcast(fp32r),
                                     mov.bitcast(fp32r),
                                     start=(w_i == 0), stop=(w_i == 5))
                ctxs = cx_pool.tile([S + 1, WIN], fp32)
                nc.vector.tensor_copy(out=ctxs[:], in_=ctxT[:])
                return (c, ctxs)

            def emit_fin(state):
                c, ctxs = state
                tout = pv_pool.tile([128, 2, S + 1], fp32, tag="ctx")
                for qh in range(2):
                    nc.tensor.transpose(tout[:, qh, :],
                                        ctxs[:, qh * 128:(qh + 1) * 128],
                                        ident_sb[:])
                rc = sm_pool.tile([128, 2, 1], fp32)
                nc.vector.reciprocal(rc[:], tout[:, :, S:S + 1])
                ob = o_pool.tile([128, 2, S], fp32)
                for qh in range(2):
                    nc.vector.tensor_scalar_mul(ob[:, qh, :], tout[:, qh, 0:S],
                                                rc[:, qh, 0:1])
                nc.sync.dma_start(
                    out=out.ap()[c * WIN:(c + 1) * WIN, g * S:(g + 1) * S]
                        .rearrange("(q2 p) s -> p q2 s", p=128),
                    in_=ob[:])

            if "attn" in parts:
                stages = []
                for c in range(C):
                    stages.append(emit_qk(c))
                    if len(stages) >= 2:
                        stages[-2] = emit_pv(stages[-2])
                    if len(stages) >= 3:
                        emit_fin(stages.pop(0))
                stages[-1] = emit_pv(stages[-1])
                for s2 in stages:
                    emit_fin(s2)

    with TileContext(nc) as tc:
        with ExitStack() as ctx:
            if loop_n is None:
                emit(tc, ctx)
            else:
                with tc.For_i(0, loop_n, 1):
                    emit(tc, ctx)
    nc.compile()
    return nc


def _make_masks():
    """0/1 multiplicative masks for the zero-padded prev/next blocks, in
    expS.T layout [key_within_tile, q]. Slots 0,1: chunk-0 prev tiles;
    slots 2,3: chunk-15 next tiles."""
    m = np.ones((128, 4, WIN), dtype=np.float32)
    p = np.arange(128)[:, None]
    q = np.arange(WIN)[None, :]
    for kt in range(2):
        k = kt * 128 + p
        m[:, kt, :] = np.where(q < WIN - k, 0.0, 1.0)
    for et in range(2):
        kn = et * 128 + p
        m[:, 2 + et, :] = np.where(q >= (WIN - 1) - kn, 0.0, 1.0)
    return m


def _prep_inputs(x, wq, wk, wv):
    """Host-side shard prep: per-core input dicts."""
    masks = _make_masks()
    xTb = [np.ascontiguousarray(x[b].T) for b in range(B)]
    wqs = wq.astype(np.float32) * np.float32(1.0 / math.sqrt(S))
    in_maps = []
    for core in range(NCORES):
        b, grp = divmod(core, 4)
        h0 = grp * NH
        wqk_np = np.empty((E, NH, 128), dtype=np.float32)
        wv_np = np.zeros((E, 4 * S), dtype=np.float32)
        for g in range(NH):
            h = h0 + g
            wqk_np[:, g, 0:64] = wqs[h * S:(h + 1) * S, :].T
            wqk_np[:, g, 64:128] = wk[h * S:(h + 1) * S, :].T
            wv_np[:, g * S:(g + 1) * S] = wv[h * S:(h + 1) * S, :].T
        in_maps.append({"xT": xTb[b], "wqk": wqk_np, "wv": wv_np,
                        "masks": masks,
                        "ident": np.eye(S + 1, dtype=np.float32)})
    return in_maps


class _Runner:
    """Compile once; execute many times via PJRT across the 8 cores."""

    def __init__(self, loop_n=None):
        import jax
        import concourse.mybir as mybir
        from concourse import bass2jax
        from jax.sharding import Mesh, PartitionSpec
        from jax.experimental.shard_map import shard_map

        self.jax = jax
        nc = _build_module(loop_n=loop_n)
        self.nc = nc
        bass2jax.install_neuronx_cc_hook()

        partition_name = (nc.partition_id_tensor.name
                          if nc.partition_id_tensor else None)
        in_names, out_names, out_avals = [], [], []
        for alloc in nc.m.functions[0].allocations:
            if not isinstance(alloc, mybir.MemoryLocationSet):
                continue
            name = alloc.memorylocations[0].name
            if alloc.kind == "ExternalInput":
                if name != partition_name:
                    in_names.append(name)
            elif alloc.kind == "ExternalOutput":
                out_names.append(name)
                out_avals.append(jax.core.ShapedArray(
                    tuple(alloc.tensor_shape), mybir.dt.np(alloc.dtype)))
        self.in_names = in_names
        self.out_names = out_names
        n_params = len(in_names)
        n_outs = len(out_names)
        self.out_avals = out_avals
        in_names_all = list(in_names) + list(out_names)
        if partition_name:
            in_names_all.append(partition_name)

        def _body(*args):
            operands = list(args)
            if partition_name is not None:
                operands.append(bass2jax.partition_id_tensor())
            outs = bass2jax._bass_exec_p.bind(
                *operands, out_avals=tuple(out_avals),
                in_names=tuple(in_names_all), out_names=tuple(out_names),
                lowering_input_output_aliases=(),
                sim_require_finite=True, sim_require_nnan=True, nc=nc)
            return tuple(outs)

        devices = jax.devices()[:NCORES]
        mesh = Mesh(np.asarray(devices), ("core",))
        self._fn = jax.jit(
            shard_map(_body, mesh=mesh,
                      in_specs=(PartitionSpec("core"),) * (n_params + n_outs),
                      out_specs=(PartitionSpec("core"),) * n_outs,
                      check_rep=False),
            keep_unused=True)

    def put_args(self, in_maps):
        concat_in = [np.concatenate([m[nm] for m in in_maps], axis=0)
                     for nm in self.in_names]
        concat_zero = [np.zeros((NCORES * a.shape[0], *a.shape[1:]), a.dtype)
                       for a in self.out_avals]
        return [self.jax.device_put(a) for a in concat_in + concat_zero]

    def run(self, args):
        res = self.jax.block_until_ready(self._fn(*args))
        return [np.asarray(r) for r in res]


_RUNNER = None


def kernel(x, wq, wk, wv):
    global _RUNNER
    x = np.asarray(x, dtype=np.float32)
    wq = np.asarray(wq, dtype=np.float32)
    wk = np.asarray(wk, dtype=np.float32)
    wv = np.asarray(wv, dtype=np.float32)
    if _RUNNER is None:
        _RUNNER = _Runner()
    in_maps = _prep_inputs(x, wq, wk, wv)
    args = _RUNNER.put_args(in_maps)
    outs = _RUNNER.run(args)
    o = outs[0].reshape(NCORES, T, NH * S)
    full = np.empty((B, T, E), dtype=np.float32)
    for core in range(NCORES):
        b, grp = divmod(core, 4)
        full[b, :, grp * NH * S:(grp + 1) * NH * S] = o[core]
    return full


# revision 21
# speedup vs baseline: 9.1313x; 1.0446x over previous
"""Longformer-style sliding-chunk self-attention for Trainium2 (Bass/Tile).

Problem: B=2, T=4096, E=768, H=12 heads (head dim 64), window chunk W=256.
  q = (x @ wq.T)/8, k = x @ wk.T, v = x @ wv.T  (per head)
  scores: each chunk of 256 queries attends to [prev, cur, next] chunks
  (3*256 = 768 keys, zero-padded at sequence ends, with triangular masks on
  the pad blocks), softmax over the 768 window, then probs @ V.

Sharding: 8 cores = 2 batches x 4 head-groups of 3 heads. Each core gets
x[b].T (pre-transposed on host), per-head weight slices (transposed on
host, with the 1/8 query scale folded into wq), and produces
out[b, :, g*192:(g+1)*192].

Per-core kernel (all layouts chosen so no on-chip transposes are needed):
  - Q.T, K.T computed in [head_dim, T] layout (PSUM out of matmuls with
    weight slices as the stationary operand, x.T streaming).
  - V computed in natural [T, head_dim] layout (x.T tiles stationary,
    wv.T streaming), stored with a ones-column appended (V_aug) so the
    P@V matmul also produces the softmax denominator for free.
  - scores are computed TRANSPOSED: S.T[key, q] = K.T_tile.T @ Q.T_chunk,
    softmax uses exp WITHOUT max subtraction (scores ~ N(0,1), max < ~7,
    exp is safe in fp32) so no partition-dim reduction is ever needed.
  - P@V runs transposed: ctx.T[s|denom, q] = V_aug.T @ expS.T with V_aug
    [k,65] stationary (cheap weight load) and expS.T [k,256] moving
    (full-rate float32r streaming); row 64 is the softmax denominator.
    Two PE transposes (65x128 -> 128x65) restore [q, s] layout, then
    reciprocal + tensor_scalar multiply normalize, and the result DMAs out.
  - boundary chunks: the zero-padded prev/next blocks have score 0, so
    exp(0)*mask = mask; the 0/1 mask tiles (precomputed on host) are used
    directly as the "expS" moving operand with a zeros+ones V_aug pad
    tile, which also fixes the denominator. No masking work in the kernel.
  - matmuls use float32r (fp32 via bf16 hi/lo replication in the PE):
    1 cycle/row when the moving dim >= 256 vs 4 cycles/row for plain fp32.
    Rel err vs the fp32 reference is ~4.6e-4 (plain fp32 was ~4e-6).
  - the attention loop is software-pipelined 3 deep (QK(c) | PV(c-1) |
    transpose+normalize(c-2)) because the PE executes in order and would
    otherwise stall on the ACT exp between QK(c) and PV(c).
"""

import math

import numpy as np

B, T, E, H, WIN = 2, 4096, 768, 12, 256
S = 64            # head dim
NH = 3            # heads per core
ET = E // 128     # 6 e-tiles
TT = T // 128     # 32 t-tiles
C = T // WIN      # 16 chunks
NCORES = 8
NCH = 8           # 512-wide column chunks for the projections
PROJN = T // NCH  # 512


def _build_module(loop_n=None, parts=("load", "vproj", "qkproj", "attn")):
    """Build + compile the per-core Bass module. Same program on all cores.

    parts: ablation control for timing experiments (kernel() always uses all).
    """
    from contextlib import ExitStack

    import concourse.mybir as mybir
    from concourse import bacc
    from concourse.tile import TileContext

    fp32 = mybir.dt.float32
    fp32r = mybir.dt.float32r
    Exp = mybir.ActivationFunctionType.Exp

    nc = bacc.Bacc("TRN2", target_bir_lowering=False, debug=False,
                   num_devices=NCORES)
    xT = nc.dram_tensor("xT", [E, T], fp32, kind="ExternalInput")
    wqk = nc.dram_tensor("wqk", [E, NH, 128], fp32, kind="ExternalInput")
    # wv is padded to 4*S=256 columns (last 64 zero) so the V projection's
    # moving dim is 256, which lets float32r run at 1 cycle/row.
    wv = nc.dram_tensor("wv", [E, 4 * S], fp32, kind="ExternalInput")
    masks = nc.dram_tensor("masks", [128, 4, WIN], fp32, kind="ExternalInput")
    ident = nc.dram_tensor("ident", [S + 1, S + 1], fp32, kind="ExternalInput")
    out = nc.dram_tensor("out", [T, NH * S], fp32, kind="ExternalOutput")

    def emit(tc, ctx):
        singles = ctx.enter_context(tc.tile_pool(name="singles", bufs=1))
        qk_pool = ctx.enter_context(tc.tile_pool(name="qk", bufs=1))
        st_pool = ctx.enter_context(tc.tile_pool(name="st", bufs=2, space="PSUM"))
        pv_pool = ctx.enter_context(tc.tile_pool(name="pv", bufs=2, space="PSUM"))
        ex_pool = ctx.enter_context(tc.tile_pool(name="ex", bufs=2))
        o_pool = ctx.enter_context(tc.tile_pool(name="o", bufs=4))
        cx_pool = ctx.enter_context(tc.tile_pool(name="cx", bufs=3))
        sm_pool = ctx.enter_context(tc.tile_pool(name="sm", bufs=4))

        # ---- persistent SBUF tensors ----
        xt = singles.tile([128, ET, T], fp32)            # x[b].T   96KB/part
        wqk_sb = singles.tile([128, ET, NH, 128], fp32)  # 9KB/part
        wv_sb = singles.tile([128, ET, 4 * S], fp32)     # 6KB/part
        mask_sb = singles.tile([128, 4, WIN], fp32)      # 4KB/part
        v3 = singles.tile([128, TT, NH, S + 1], fp32)    # V_aug  24.4KB/part
        vpad = singles.tile([128, S + 1], fp32)
        ident_sb = singles.tile([S + 1, S + 1], fp32)

        # ---- input loads ----
        # xt/wqk/wv feed float32r matmuls: the BIR verifier requires their
        # producers to emit float32r, so the loads are bitcast on both sides
        # (same 4-byte values; the PE does the hi/lo bf16 split at load).
        xT_r = xT.ap().bitcast(fp32r).rearrange("(a p) t -> a p t", p=128)
        if "load" in parts:
            for tq in range(4):
                for et in range(ET):
                    nc.sync.dma_start(out=xt[:, et, tq * 1024:(tq + 1) * 1024].bitcast(fp32r),
                                      in_=xT_r[et][:, tq * 1024:(tq + 1) * 1024])
        nc.sync.dma_start(out=wqk_sb[:].bitcast(fp32r),
                          in_=wqk.ap().bitcast(fp32r).rearrange("(a p) g m -> p a g m", p=128))
        nc.sync.dma_start(out=wv_sb[:].bitcast(fp32r),
                          in_=wv.ap().bitcast(fp32r).rearrange("(a p) m -> p a m", p=128))
        nc.sync.dma_start(out=mask_sb[:].bitcast(fp32r),
                          in_=masks.ap().bitcast(fp32r))
        nc.sync.dma_start(out=ident_sb[:], in_=ident.ap())
        # DVE memset cannot emit float32r; stage the constants in fp32 and
        # round through ACT copies (valid float32r producers).
        cst = singles.tile([128, S + 1 + TT * NH], fp32)
        nc.vector.memset(cst[:], 0.0)
        nc.vector.memset(cst[:, S:S + 1], 1.0)
        nc.vector.memset(cst[:, S + 1:], 1.0)
        nc.scalar.copy(out=vpad[:].bitcast(fp32r), in_=cst[:, 0:S + 1])
        nc.scalar.copy(
            out=v3[:, :, :, S:S + 1].bitcast(fp32r),
            in_=cst[:, S + 1:].rearrange("p (a g one) -> p a g one",
                                         a=TT, g=NH, one=1))

        # ---- V projection, all heads: V[t, s] (+ ones col) ----
        for tt in range(TT if "vproj" in parts else 0):
            pvv = pv_pool.tile([128, 4 * S], fp32, tag="ctx")
            for et in range(ET):
                nc.tensor.matmul(pvv[:],
                                 xt[:, et, tt * 128:(tt + 1) * 128].bitcast(fp32r),
                                 wv_sb[:, et, :].bitcast(fp32r),
                                 start=(et == 0), stop=(et == ET - 1))
            nc.vector.tensor_copy(
                out=v3[:, tt, :, 0:S].bitcast(fp32r),
                in_=pvv[:, 0:NH * S].rearrange("p (g s) -> p g s", g=NH))

        # ---- per-head: Q.T/K.T projection, then attention ----
        for g in range(NH):
            qt = qk_pool.tile([64, T], fp32, tag="qt")
            kt = qk_pool.tile([64, T], fp32, tag="kt")
            if g == 0 and "attn" in parts and "qkproj" not in parts:
                nc.vector.memset(qt[:], 0.01)
                nc.vector.memset(kt[:], 0.01)
            for nch in range(NCH if "qkproj" in parts else 0):
                sl = slice(nch * PROJN, (nch + 1) * PROJN)
                psq = pv_pool.tile([64, PROJN], fp32, tag="ctx")
                for et in range(ET):
                    nc.tensor.matmul(psq[:], wqk_sb[:, et, g, 0:64].bitcast(fp32r),
                                     xt[:, et, sl].bitcast(fp32r),
                                     start=(et == 0), stop=(et == ET - 1))
                nc.scalar.copy(out=qt[:, sl].bitcast(fp32r), in_=psq[:])
                psk = pv_pool.tile([64, PROJN], fp32, tag="ctx")
                for et in range(ET):
                    nc.tensor.matmul(psk[:], wqk_sb[:, et, g, 64:128].bitcast(fp32r),
                                     xt[:, et, sl].bitcast(fp32r),
                                     start=(et == 0), stop=(et == ET - 1))
                nc.vector.tensor_copy(out=kt[:, sl].bitcast(fp32r), in_=psk[:])

            # Software-pipelined attention: the PE is in-order, so PV(c) right
            # after QK(c) would stall on exp(c). Emit QK(c+1) between exp(c)
            # and PV(c) so the PE always has independent matmuls in hand.
            def emit_qk(c):
                lo = 2 if c == 0 else 0        # first valid window key-tile
                hi = 4 if c == C - 1 else 6    # one past last valid
                stp = st_pool.tile([128, 6, WIN], fp32, tag="st")
                for w_i in range(lo, hi):
                    gk = (c - 1) * 2 + w_i
                    nc.tensor.matmul(stp[:, w_i, :],
                                     kt[:, gk * 128:(gk + 1) * 128].bitcast(fp32r),
                                     qt[:, c * WIN:(c + 1) * WIN].bitcast(fp32r),
                                     start=True, stop=True)
                ex = ex_pool.tile([128, 6, WIN], fp32)
                nc.scalar.activation(out=ex[:, lo:hi, :].bitcast(fp32r),
                                     in_=stp[:, lo:hi, :], func=Exp)
                return (c, lo, hi, ex)

            def emit_pv(state):
                # ctx.T[s|denom, q] = sum_k V_aug[k, s] * expS.T[k, q]:
                # V_aug stationary (65 cols -> cheap weight load), expS.T
                # moving (256 -> full-rate fp32r streaming).
                c, lo, hi, ex = state
                ctxT = pv_pool.tile([S + 1, WIN], fp32, tag="ctx")
                for w_i in range(6):
                    if w_i < lo:
                        sta, mov = vpad[:], mask_sb[:, w_i, :]
                    elif w_i >= hi:
                        sta, mov = vpad[:], mask_sb[:, 2 + (w_i - 4), :]
                    else:
                        gk = (c - 1) * 2 + w_i
                        sta, mov = v3[:, gk, g, :], ex[:, w_i, :]
                    nc.tensor.matmul(ctxT[:], sta.bitcast(fp32r),
                                     mov.bitcast(fp32r),
                                     start=(w_i == 0), stop=(w_i == 5))
                ctxs = cx_pool.tile([S + 1, WIN], fp32)
                nc.vector.tensor_copy(out=ctxs[:], in_=ctxT[:])
                return (c, ctxs)

            def emit_fin(state):
                c, ctxs = state
                tout = pv_pool.tile([128, 2, S + 1], fp32, tag="ctx")
                for qh in range(2):
                    nc.tensor.transpose(tout[:, qh, :],
                                        ctxs[:, qh * 128:(qh + 1) * 128],
                                        ident_sb[:])
                rc = sm_pool.tile([128, 2, 1], fp32)
                nc.vector.reciprocal(rc[:], tout[:, :, S:S + 1])
                ob = o_pool.tile([128, 2, S], fp32)
                for qh in range(2):
                    nc.vector.tensor_scalar_mul(ob[:, qh, :], tout[:, qh, 0:S],
                                                rc[:, qh, 0:1])
                nc.sync.dma_start(
                    out=out.ap()[c * WIN:(c + 1) * WIN, g * S:(g + 1) * S]
                        .rearrange("(q2 p) s -> p q2 s", p=128),
                    in_=ob[:])

            if "attn" in parts:
                stages = []
                for c in range(C):
                    stages.append(emit_qk(c))
                    if len(stages) >= 2:
                        stages[-2] = emit_pv(stages[-2])
                    if len(stages) >= 3:
                        emit_fin(stages.pop(0))
                stages[-1] = emit_pv(stages[-1])
                for s2 in stages:
                    emit_fin(s2)

    with TileContext(nc) as tc:
        with ExitStack() as ctx:
            if loop_n is None:
                emit(tc, ctx)
            else:
                with tc.For_i(0, loop_n, 1):
                    emit(tc, ctx)
    nc.compile()
    return nc


def _make_masks():
    """0/1 multiplicative masks for the zero-padded prev/next blocks, in
    expS.T layout [key_within_tile, q]. Slots 0,1: chunk-0 prev tiles;
    slots 2,3: chunk-15 next tiles."""
    m = np.ones((128, 4, WIN), dtype=np.float32)
    p = np.arange(128)[:, None]
    q = np.arange(WIN)[None, :]
    for kt in range(2):
        k = kt * 128 + p
        m[:, kt, :] = np.where(q < WIN - k, 0.0, 1.0)
    for et in range(2):
        kn = et * 128 + p
        m[:, 2 + et, :] = np.where(q >= (WIN - 1) - kn, 0.0, 1.0)
    return m


def _prep_inputs(x, wq, wk, wv):
    """Host-side shard prep: per-core input dicts."""
    masks = _make_masks()
    xTb = [np.ascontiguousarray(x[b].T) for b in range(B)]
    wqs = wq.astype(np.float32) * np.float32(1.0 / math.sqrt(S))
    in_maps = []
    for core in range(NCORES):
        b, grp = divmod(core, 4)
        h0 = grp * NH
        wqk_np = np.empty((E, NH, 128), dtype=np.float32)
        wv_np = np.zeros((E, 4 * S), dtype=np.float32)
        for g in range(NH):
            h = h0 + g
            wqk_np[:, g, 0:64] = wqs[h * S:(h + 1) * S, :].T
            wqk_np[:, g, 64:128] = wk[h * S:(h + 1) * S, :].T
            wv_np[:, g * S:(g + 1) * S] = wv[h * S:(h + 1) * S, :].T
        in_maps.append({"xT": xTb[b], "wqk": wqk_np, "wv": wv_np,
                        "masks": masks,
                        "ident": np.eye(S + 1, dtype=np.float32)})
    return in_maps


class _Runner:
    """Compile once; execute many times via PJRT across the 8 cores."""

    def __init__(self, loop_n=None):
        import jax
        import concourse.mybir as mybir
        from concourse import bass2jax
        from jax.sharding import Mesh, PartitionSpec
        from jax.experimental.shard_map import shard_map

        self.jax = jax
        nc = _build_module(loop_n=loop_n)
        self.nc = nc
        bass2jax.install_neuronx_cc_hook()

        partition_name = (nc.partition_id_tensor.name
                          if nc.partition_id_tensor else None)
        in_names, out_names, out_avals = [], [], []
        for alloc in nc.m.functions[0].allocations:
            if not isinstance(alloc, mybir.MemoryLocationSet):
                continue
            name = alloc.memorylocations[0].name
            if alloc.kind == "ExternalInput":
                if name != partition_name:
                    in_names.append(name)
            elif alloc.kind == "ExternalOutput":
                out_names.append(name)
                out_avals.append(jax.core.ShapedArray(
                    tuple(alloc.tensor_shape), mybir.dt.np(alloc.dtype)))
        self.in_names = in_names
        self.out_names = out_names
        n_params = len(in_names)
        n_outs = len(out_names)
        self.out_avals = out_avals
        in_names_all = list(in_names) + list(out_names)
        if partition_name:
            in_names_all.append(partition_name)

        def _body(*args):
            operands = list(args)
            if partition_name is not None:
                operands.append(bass2jax.partition_id_tensor())
            outs = bass2jax._bass_exec_p.bind(
                *operands, out_avals=tuple(out_avals),
                in_names=tuple(in_names_all), out_names=tuple(out_names),
                lowering_input_output_aliases=(),
                sim_require_finite=True, sim_require_nnan=True, nc=nc)
            return tuple(outs)

        devices = jax.devices()[:NCORES]
        mesh = Mesh(np.asarray(devices), ("core",))
        self._fn = jax.jit(
            shard_map(_body, mesh=mesh,
                      in_specs=(PartitionSpec("core"),) * (n_params + n_outs),
                      out_specs=(PartitionSpec("core"),) * n_outs,
                      check_rep=False),
            keep_unused=True)

    def put_args(self, in_maps):
        concat_in = [np.concatenate([m[nm] for m in in_maps], axis=0)
                     for nm in self.in_names]
        concat_zero = [np.zeros((NCORES * a.shape[0], *a.shape[1:]), a.dtype)
                       for a in self.out_avals]
        return [self.jax.device_put(a) for a in concat_in + concat_zero]

    def run(self, args):
        res = self.jax.block_until_ready(self._fn(*args))
        return [np.asarray(r) for r in res]


_RUNNER = None


def kernel(x, wq, wk, wv):
    global _RUNNER
    x = np.asarray(x, dtype=np.float32)
    wq = np.asarray(wq, dtype=np.float32)
    wk = np.asarray(wk, dtype=np.float32)
    wv = np.asarray(wv, dtype=np.float32)
    if _RUNNER is None:
        _RUNNER = _Runner()
    in_maps = _prep_inputs(x, wq, wk, wv)
    args = _RUNNER.put_args(in_maps)
    outs = _RUNNER.run(args)
    o = outs[0].reshape(NCORES, T, NH * S)
    full = np.empty((B, T, E), dtype=np.float32)
    for core in range(NCORES):
        b, grp = divmod(core, 4)
        full[b, :, grp * NH * S:(grp + 1) * NH * S] = o[core]
    return full
